# revision 48
# baseline (speedup 1.0000x reference)
"""2-layer GCN (GCNConv x2, symmetric norm, self-loops) on 8 Trainium2 NeuronCores.

Strategy (graph/data parallel) — chunked collectives + ragged chunks:
  - Nodes are partitioned contiguously across 8 cores (6250/core, padded to
    6272 = 49*128). Within a core, nodes are permuted into 49 blocks of 128
    with greedy in-degree balancing (two rounds, since an edge's src group
    depends on the src node's block assignment).
  - Blocks are split into G=2 groups. The gather tables are per-group Shared
    DRAM tiles (h_full_g / g_full_g, <= 25600 rows so dma_gather's int16
    indices cover them); each AllGather is a per-group collective issued as
    soon as that group's rows are computed, overlapping remaining compute.
  - Edge slots are grouped by (dst block, src group) with RAGGED chunk
    counts: chunk count for (b, g) is the max over the 8 cores of that
    bucket's edge count (a compile-time constant, so the SPMD program is
    shared), not a global max — measured 911 chunks/core/phase vs 980 for
    a global split (7% less gather traffic / M work / matmuls).
  - Layer 1 dense transform h = x @ W1 is row-sharded: each core multiplies
    its [6272, 4096] x-slice (fed pre-transposed, bf16) against replicated
    W1; AG1 chunk g is issued right after group g's blocks are stored.
  - Aggregation: dma_gather fetches h[src] rows per edge slot; a one-hot
    selection matrix M scatter-adds them on the TensorEngine with PSUM
    accumulation. norm = dinv[src]*dinv[dst] is folded into M, so padded
    slots (norm=0) are inert. M tiles are built ~70% on DVE (tensor_scalar
    is_equal*mult) and ~30% on ACT (|mdst-iota| then relu(norm - norm*t),
    exact for the integer grid) — both engines run ~2ns/col, so splitting
    the ~1 op / 128 edges cost is what keeps M construction off the
    critical path.
  - bias+ReLU and the layer-2 transform @ W2 run per 128-node block (PSUM ->
    SBUF copies on the ACT engine); AG2 chunk g is issued after group g's
    blocks finish, then the same gather/scatter aggregates layer 2. Matmuls
    run in two passes per superblock (every block's g0 chunks, then g1) so
    the PE is not head-of-line blocked while an AG chunk is still landing.

kernel(**inputs) takes full unsharded inputs, returns the full [50000, 128]
output. Self-contained: no sibling imports; /opt/trn_rl_repo provides bass.
"""

import math
import sys

import numpy as np

sys.path.insert(0, "/opt/trn_rl_repo")

import concourse.bass as bass  # noqa: E402
import concourse.mybir as mybir  # noqa: E402
import concourse.tile as tile  # noqa: E402
from concourse import bacc  # noqa: E402

P = 128
NCORES = 8
SB = 3  # blocks per gather superblock
GMAX = 6  # chunks (of 128 idxs) per dma_gather op; 6*128=768 is the HW cap

F32 = mybir.dt.float32
BF16 = mybir.dt.bfloat16
I16 = mybir.dt.int16


# ---------------------------------------------------------------------------
# host-side preprocessing
# ---------------------------------------------------------------------------

def _groups(nb):
    """Block groups for chunked AllGathers (G=2: best ragged-padding vs
    collective-overlap tradeoff; more groups round more partial chunks)."""
    g = min(2, nb)
    gb = math.ceil(nb / g)
    return [list(range(s, min(s + gb, nb))) for s in range(0, nb, gb)]


def _superblocks(groups):
    sbs = []
    for g in groups:
        for s in range(0, len(g), SB):
            sbs.append(tuple(g[s:s + SB]))
    return sbs


def _swizzle_idx(idx):
    """gather idx j -> [j%16, j//16], replicated across the 8 groups of 16."""
    n = idx.shape[0]
    a = np.zeros((16, n // 16), np.int16)
    a[np.arange(n) % 16, np.arange(n) // 16] = idx.astype(np.int16)
    return np.tile(a, (8, 1))


def _swizzle_slot(v):
    """slot s -> [s%128, s//128] (dma_gather / matmul-chunk layout)."""
    n = v.shape[0]
    a = np.zeros((P, n // P), np.float32)
    a[np.arange(n) % P, np.arange(n) // P] = v.astype(np.float32)
    return a


def _pack_blocks(loads, nb):
    """Greedy bin-pack node ids (by descending load) into nb blocks of <=128.

    Returns pos[i] = block*128 + slot for each local node i."""
    import heapq

    n = loads.shape[0]
    order = np.argsort(-loads, kind="stable")
    heap = [(0.0, b) for b in range(nb)]
    heapq.heapify(heap)
    counts = np.zeros(nb, np.int64)
    pos = np.empty(n, np.int64)
    for i in order:
        while True:
            load, b = heapq.heappop(heap)
            if counts[b] < P:
                break
        pos[i] = b * P + counts[b]
        counts[b] += 1
        if counts[b] < P:
            heapq.heappush(heap, (load + loads[i], b))
    return pos


def _pack_blocks_nd(d, nb):
    """Greedy pack minimizing the per-block max over src-group loads."""
    n, G = d.shape
    tot = d.sum(axis=1)
    order = np.argsort(-tot, kind="stable")
    cur = np.zeros((nb, G))
    counts = np.zeros(nb, np.int64)
    pos = np.empty(n, np.int64)
    for i in order:
        nxt = cur + d[i]
        score = nxt.max(axis=1) + 0.001 * nxt.sum(axis=1)
        score[counts >= P] = np.inf
        b = int(np.argmin(score))
        pos[i] = b * P + counts[b]
        counts[b] += 1
        cur[b] += d[i]
    return pos


class Layout:
    """Static ragged chunk layout shared by host prep and device build.

    c_bg[b][g]: chunks for (dst block b, src group g) = max edge count over
    cores, rounded up to 128s. All derived offsets are compile-time."""

    def __init__(self, nb, groups, c_bg):
        self.nb = nb
        self.groups = groups
        self.G = len(groups)
        self.c_bg = c_bg
        self.sbs = _superblocks(groups)
        # idx table: for g: for b: c_bg[b][g] chunks (contiguous per group)
        self.idx_off = np.zeros((self.G, nb + 1), np.int64)
        off = 0
        for g in range(self.G):
            for b in range(nb):
                self.idx_off[g][b] = off
                off += c_bg[b][g]
            self.idx_off[g][nb] = off
        self.total_chunks = off
        # msg/m-data chunk order: per superblock: for g: for b in sb: chunks
        self.sb_base = {}
        self.sb_cols = {}   # (sb_index) -> dict[(g, b)] = col offset in sb
        base = 0
        for si, blocks in enumerate(self.sbs):
            cols = {}
            c = 0
            for g in range(self.G):
                for b in blocks:
                    cols[(g, b)] = c
                    c += c_bg[b][g]
            self.sb_base[si] = base
            self.sb_cols[si] = cols
            self.sb_len = None
            base += c
        self.total_mchunks = base

    def sb_width(self, si):
        blocks = self.sbs[si]
        return sum(self.c_bg[b][g] for b in blocks for g in range(self.G))

    def block_cols(self, si, b):
        """msg column indices for block b's chunks, group-ascending."""
        cols = []
        for g in range(self.G):
            c0 = self.sb_cols[si][(g, b)]
            cols.extend(range(c0, c0 + self.c_bg[b][g]))
        return cols


def _prep(x, edge_index, W1, b1, W2, b2):
    N, F_in = x.shape
    F_h = W1.shape[1]
    F_out = W2.shape[1]
    assert N % NCORES == 0 and F_in % P == 0 and F_h == 2 * P and F_out == P
    npc_raw = N // NCORES
    nb = math.ceil(npc_raw / P)
    npc = nb * P

    groups = _groups(nb)
    G = len(groups)
    gb = len(groups[0])
    glen = np.array([len(g) for g in groups])
    assert int(glen.max()) * P * NCORES < 2 ** 15
    grp_of_blk = np.arange(nb) // gb
    first_blk = np.array([g[0] for g in groups])

    src = np.concatenate([np.asarray(edge_index[0]), np.arange(N)]).astype(np.int64)
    dst = np.concatenate([np.asarray(edge_index[1]), np.arange(N)]).astype(np.int64)
    deg = np.bincount(dst, minlength=N).astype(np.float64)
    dinv = np.where(deg > 0, 1.0 / np.sqrt(deg), 0.0)
    norm = (dinv[src] * dinv[dst]).astype(np.float32)

    core_dst = dst // npc_raw

    # --- round 1: pack by total in-degree to get provisional src groups ---
    pos1 = np.empty(N, np.int64)
    for c in range(NCORES):
        nodes = np.arange(c * npc_raw, (c + 1) * npc_raw)
        pos1[nodes] = _pack_blocks(deg[nodes], nb)
    g1 = grp_of_blk[pos1[src] // P]

    # --- round 2: pack balancing per-src-group in-degree ---
    d_by_g = np.zeros((N, G))
    np.add.at(d_by_g, (dst, g1), 1)
    pos = np.empty(N, np.int64)
    for c in range(NCORES):
        nodes = np.arange(c * npc_raw, (c + 1) * npc_raw)
        pos[nodes] = _pack_blocks_nd(d_by_g[nodes], nb)

    # final groups and in-group rows from the round-2 packing. Group 0's
    # table gets one dummy block per core: its dummy rows in h_own_0 are
    # written at the END of phase A so AG1_0 cannot start early and steal
    # HBM bandwidth from the x@W1 stream (which HAM-rethrottles the PE);
    # g_own_0's dummy rows are written at the START of phase C so AG2_0
    # still fires as early as possible.
    pad_blk = np.zeros(G, np.int64)
    if G > 1:
        pad_blk[0] = 1
    assert int((glen + pad_blk).max()) * P * NCORES < 2 ** 15
    blk = pos // P
    slot = pos % P
    grp = grp_of_blk[blk]
    core_of = np.arange(N) // npc_raw
    row_in_grp = (core_of * (glen + pad_blk)[grp]
                  + (blk - first_blk[grp])) * P + slot
    e_grp = grp[src]
    e_row = row_in_grp[src]

    # per (core, block, group) edge counts -> ragged chunk constants
    blk_of_dst = blk[dst]
    cnt = np.zeros((NCORES, nb, G), np.int64)
    np.add.at(cnt, (core_dst, blk_of_dst, e_grp), 1)
    c_bg = np.ceil(cnt.max(axis=0) / P).astype(np.int64)  # [nb, G]
    lay = Layout(nb, groups, c_bg)

    cores = []
    for c in range(NCORES):
        mask = core_dst == c
        e_src_row = e_row[mask]
        e_g = e_grp[mask]
        e_blk = blk_of_dst[mask]
        e_dl = (pos[dst[mask]] % P).astype(np.float32)
        e_nrm = norm[mask]

        idx_flat = np.zeros(lay.total_chunks * P, np.int64)
        dl_by_chunk = {}   # (g, b) -> [c_bg, P]
        nrm_by_chunk = {}
        for g in range(G):
            sel = e_g == g
            sr = e_src_row[sel]
            bl = e_blk[sel]
            order = np.argsort(bl, kind="stable")
            sr, bl = sr[order], bl[order]
            dl_s, nr_s = e_dl[sel][order], e_nrm[sel][order]
            start = np.searchsorted(bl, np.arange(nb))
            end = np.searchsorted(bl, np.arange(nb) + 1)
            for b in range(nb):
                c_g = int(c_bg[b][g])
                k = end[b] - start[b]
                assert k <= c_g * P, (k, c_g)
                sl = slice(start[b], end[b])
                o0 = lay.idx_off[g][b] * P
                idx_flat[o0: o0 + k] = sr[sl]
                flat_dl = np.zeros(c_g * P, np.float32)
                flat_nr = np.zeros(c_g * P, np.float32)
                flat_dl[:k] = dl_s[sl]
                flat_nr[:k] = nr_s[sl]
                dl_by_chunk[(g, b)] = flat_dl.reshape(c_g, P)
                nrm_by_chunk[(g, b)] = flat_nr.reshape(c_g, P)
            seg = idx_flat[lay.idx_off[g][0] * P: lay.idx_off[g][nb] * P]
            assert seg.min() >= 0
            assert seg.max(initial=0) < (glen[g] + pad_blk[g]) * P * NCORES

        # mdata in device chunk order: per superblock: for g: for b: chunks
        md, mn = [], []
        for si, blocks in enumerate(lay.sbs):
            for g in range(G):
                for b in blocks:
                    md.append(dl_by_chunk[(g, b)])
                    mn.append(nrm_by_chunk[(g, b)])
        mdst = np.concatenate(md).reshape(lay.total_mchunks * P)
        mnorm = np.concatenate(mn).reshape(lay.total_mchunks * P)

        # x slice, permuted and transposed: xt[f, pos] = x[node, f]
        nodes = np.arange(c * npc_raw, (c + 1) * npc_raw)
        xp = np.zeros((npc, F_in), np.float32)
        xp[pos[nodes]] = np.asarray(x[nodes], np.float32)
        xt = np.ascontiguousarray(xp.T.astype(np.float32))
        import ml_dtypes
        xt = xt.astype(ml_dtypes.bfloat16)

        cores.append({
            "xt": xt,
            "idxs": _swizzle_idx(idx_flat),
            "mdst": _swizzle_slot(mdst),
            "mnorm": _swizzle_slot(mnorm),
            "mnormneg": _swizzle_slot(-mnorm),
        })

    import ml_dtypes
    iota = np.tile(np.arange(P, dtype=np.float32)[None, :], (P, 1))
    shared = {
        "w1": ml_dtypes.bfloat16(np.asarray(W1, np.float32)),
        "w2": np.asarray(W2, np.float32),
        "b1p": np.asarray(b1, np.float32).reshape(2, P).T.copy(),
        "b2b": np.tile(np.asarray(b2, np.float32)[None, :], (P, 1)),
        "iota": iota.astype(ml_dtypes.bfloat16),
    }
    cfg = dict(N=N, F_in=F_in, F_h=F_h, F_out=F_out, npc_raw=npc_raw, nb=nb,
               npc=npc, c_bg=c_bg, pos=pos)
    return cfg, cores, shared


# ---------------------------------------------------------------------------
# device kernel
# ---------------------------------------------------------------------------

def _build_nc(cfg):
    F_in, F_h, F_out = cfg["F_in"], cfg["F_h"], cfg["F_out"]
    nb, npc = cfg["nb"], cfg["npc"]
    c_bg = cfg["c_bg"]
    kt = F_in // P
    groups = _groups(nb)
    G = len(groups)
    lay = Layout(nb, groups, c_bg)
    nchunks = lay.total_mchunks
    rg = [list(range(NCORES))]

    nc = bacc.Bacc(None, num_devices=NCORES, num_swdge_queues=4)

    xt_d = nc.declare_dram_parameter("xt", [F_in, npc], BF16, isOutput=False)
    w1_d = nc.declare_dram_parameter("w1", [F_in, F_h], BF16, isOutput=False)
    w2_d = nc.declare_dram_parameter("w2", [F_h, F_out], F32, isOutput=False)
    b1_d = nc.declare_dram_parameter("b1p", [P, 2], F32, isOutput=False)
    b2_d = nc.declare_dram_parameter("b2b", [P, F_out], F32, isOutput=False)
    iota_d = nc.declare_dram_parameter("iota", [P, P], BF16, isOutput=False)
    idxs_d = nc.declare_dram_parameter("idxs", [P, lay.total_chunks * 8], I16,
                                       isOutput=False)
    mdst_d = nc.declare_dram_parameter("mdst", [P, nchunks], F32, isOutput=False)
    mnorm_d = nc.declare_dram_parameter("mnorm", [P, nchunks], F32, isOutput=False)
    mneg_d = nc.declare_dram_parameter("mnormneg", [P, nchunks], F32,
                                       isOutput=False)
    out_d = nc.declare_dram_parameter("out", [npc, F_out], F32, isOutput=True)

    with tile.TileContext(nc) as tc:
        with (
            tc.tile_pool(name="const", bufs=1) as const,
            tc.tile_pool(name="work", bufs=1) as work,
            tc.tile_pool(name="dram", bufs=1, space="DRAM") as dram,
        ):
            # Group 0 gets one dummy block per core (see _prep): its h rows
            # are written at the end of phase A to postpone AG1_0.
            pad = [1 if (i == 0 and G > 1) else 0 for i in range(G)]
            h_own = [dram.tile([(len(g) + pad[i]) * P, F_h], BF16,
                               name=f"h_own{i}")
                     for i, g in enumerate(groups)]
            h_full = [dram.tile([(len(g) + pad[i]) * P * NCORES, F_h], BF16,
                                addr_space="Shared", name=f"h_full{i}")
                      for i, g in enumerate(groups)]
            g_own = [dram.tile([(len(g) + pad[i]) * P, F_out], BF16,
                               name=f"g_own{i}")
                     for i, g in enumerate(groups)]
            g_full = [dram.tile([(len(g) + pad[i]) * P * NCORES, F_out], BF16,
                                addr_space="Shared", name=f"g_full{i}")
                      for i, g in enumerate(groups)]

            w1_t = const.tile([P, kt, F_h], BF16)
            w2_t = const.tile([P, 2, F_out], F32)
            b1_t = const.tile([P, 2], F32)
            b2_t = const.tile([P, F_out], F32)
            iota_t = const.tile([P, P], BF16)
            idxs_t = const.tile([P, lay.total_chunks * 8], I16)
            mdst_t = const.tile([P, nchunks], F32)
            mnorm_t = const.tile([P, nchunks], F32)
            mneg_t = const.tile([P, nchunks], F32)

            nc.sync.dma_start(w1_t[:], w1_d[:].rearrange("(a p) o -> p a o", p=P))
            nc.sync.dma_start(w2_t[:], w2_d[:].rearrange("(h p) o -> p h o", p=P))
            nc.sync.dma_start(b1_t[:], b1_d[:])
            nc.sync.dma_start(b2_t[:], b2_d[:])
            nc.sync.dma_start(iota_t[:], iota_d[:])
            nc.sync.dma_start(idxs_t[:], idxs_d[:])
            nc.sync.dma_start(mdst_t[:], mdst_d[:])
            nc.sync.dma_start(mnorm_t[:], mnorm_d[:])
            nc.sync.dma_start(mneg_t[:], mneg_d[:])

            # ---- phase A: h = x @ W1, AG1 chunk per block group ----
            xt_r = xt_d[:].rearrange("(a p) n -> p a n", p=P)
            # Sub-groups of 8 blocks: xt DMAs read 8*128 contiguous columns
            # (2 KB/partition descriptors, ~300 GB/s); PSUM holds two blocks
            # per bank ([P, 2, F_h] f32 = one 2 KB bank), 4 banks per
            # sub-group, bufs=8 = all 8 banks double-buffered.
            GA = 8
            psumA = tc.tile_pool(name="psumA", bufs=1, space="PSUM")
            psum = psumA.__enter__()
            dummy_done = False
            for gi, grp in enumerate(groups):
                for s0 in range(0, len(grp), GA):
                    gblk = grp[s0:s0 + GA]
                    g0 = gblk[0]
                    phs = [psum.tile([P, F_h], F32, tag="ph", bufs=8,
                                     space="PSUM", name=f"ph{g0}_{i}")
                           for i in range(len(gblk))]
                    for a in range(kt):
                        xt_t = work.tile([P, len(gblk) * P], BF16, tag="xt",
                                         bufs=16)
                        nc.sync.dma_start(
                            xt_t[:], xt_r[:, a, g0 * P:g0 * P + len(gblk) * P])
                        for i in range(len(gblk)):
                            nc.tensor.matmul(phs[i][:],
                                             lhsT=xt_t[:, i * P:(i + 1) * P],
                                             rhs=w1_t[:, a, :],
                                             start=(a == 0), stop=(a == kt - 1))
                    for i, b in enumerate(gblk):
                        h_sb = work.tile([P, F_h], BF16, tag="hsb", bufs=3)
                        nc.vector.tensor_copy(h_sb[:], phs[i][:])
                        bw = b - grp[0]
                        nc.sync.dma_start(h_own[gi][bw * P:(bw + 1) * P, :],
                                          h_sb[:])
                    if (pad[0] and not dummy_done and gi == G - 1
                            and s0 + 3 * GA >= len(grp)):
                        # dummy block write (content irrelevant, never
                        # gathered): sequenced after ~66% of phase A's h
                        # writes so AG1_0 runs during A's last stretch and
                        # completes right around A's end, without starving
                        # the xt stream for most of A.
                        d0 = len(groups[0]) * P
                        nc.sync.dma_start(h_own[0][d0:d0 + P, :],
                                          w1_t[:, 0, :])
                        dummy_done = True
            for gi in range(G):
                nc.gpsimd.collective_compute(
                    "AllGather", mybir.AluOpType.bypass, replica_groups=rg,
                    ins=[h_own[gi][:]], outs=[h_full[gi][:]],
                )

            psumA.__exit__(None, None, None)
            psumC = tc.tile_pool(name="psumC", bufs=1, space="PSUM")
            psum = psumC.__enter__()

            qn = [0]

            def gathers(dst_t, si, tables, elem, only_g=None):
                """Per-src-group gathers for superblock si (<=GMAX chunks/op
                to stay under the 768-idx dma_gather HW cap; round-robin the
                4 SWDGE queues).

                (prepare_only/trigger_dma pipelining was tried to hide the
                ~1us per-op reissue bubble but crashes the NRT runtime on
                this stack; keep the direct path.)"""
                blocks = lay.sbs[si]
                for g in range(G):
                    if only_g is not None and g not in only_g:
                        continue
                    nch = sum(int(c_bg[b][g]) for b in blocks)
                    if nch == 0:
                        continue
                    c0 = lay.sb_cols[si][(g, blocks[0])]
                    i0 = int(lay.idx_off[g][blocks[0]])
                    for s in range(0, nch, GMAX):
                        kk = min(GMAX, nch - s)
                        nc.gpsimd.dma_gather(
                            out_ap=dst_t[:, c0 + s:c0 + s + kk, :],
                            in_ap=tables[g][:],
                            idxs_ap=idxs_t[:, (i0 + s) * 8:(i0 + s + kk) * 8],
                            num_idxs=kk * P, num_idxs_reg=kk * P,
                            elem_size=elem, queue_num=qn[0] % 4)
                        qn[0] += 1

            Abs = mybir.ActivationFunctionType.Abs
            Copy = mybir.ActivationFunctionType.Copy
            Relu = mybir.ActivationFunctionType.Relu

            def m_tile(gc, on_act=False):
                """M[s, d] = (dst[s] == d) * norm[s].

                DVE path: one is_equal*mult tensor_scalar. ACT path (~30% of
                tiles, splitting the ~2ns/col cost across both engines):
                t = |mdst - iota|; m = relu(-norm*t + norm) == one-hot*norm
                exactly on the integer grid."""
                m = work.tile([P, P], BF16, tag="m", bufs=96)
                if on_act:
                    t = work.tile([P, P], BF16, tag="mt", bufs=12)
                    nc.scalar.activation(t[:], iota_t[:], Abs,
                                         bias=mdst_t[:, gc:gc + 1], scale=-1.0)
                    nc.scalar.activation(m[:], t[:], Relu,
                                         bias=mnorm_t[:, gc:gc + 1],
                                         scale=mneg_t[:, gc:gc + 1])
                else:
                    nc.vector.tensor_scalar(
                        out=m[:], in0=iota_t[:],
                        scalar1=mdst_t[:, gc:gc + 1],
                        scalar2=mnorm_t[:, gc:gc + 1],
                        op0=mybir.AluOpType.is_equal, op1=mybir.AluOpType.mult)
                return m

            # ---- phase C: aggregate layer 1, relu, transform by W2 ----
            if pad[0]:
                # g_own_0's dummy block, written up front so AG2_0 fires as
                # soon as group 0's real blocks are done.
                d0 = len(groups[0]) * P
                nc.sync.dma_start(g_own[0][d0:d0 + P, :], iota_t[:])
            # Same FIFO-ordering trick as phase E below: pre-emit the
            # first superblocks' early-group gathers so they are not stuck
            # behind a last-group op waiting on the final AG1 chunk.
            pre_cmsgs = {}
            if G > 1:
                for si0 in range(min(3, len(lay.sbs))):
                    wp = lay.sb_width(si0)
                    pc = work.tile([P, wp, F_h], BF16, tag="msg", bufs=3,
                                   name=f"cmsg{si0}")
                    gathers(pc, si0, h_full, F_h, only_g=range(G - 1))
                    pre_cmsgs[si0] = pc
            si = 0
            for gi, grp in enumerate(groups):
                for s0 in range(0, len(grp), SB):
                    blocks = lay.sbs[si]
                    w = lay.sb_width(si)
                    gc_base = lay.sb_base[si]
                    if si in pre_cmsgs:
                        msg = pre_cmsgs[si]
                        gathers(msg, si, h_full, F_h, only_g=[G - 1])
                    else:
                        msg = work.tile([P, w, F_h], BF16, tag="msg", bufs=3)
                        gathers(msg, si, h_full, F_h)
                    # Two passes (all blocks' g0 chunks, then g1): during
                    # the AG-chunk-g1 wait the PE drains every block's g0
                    # work instead of head-of-line blocking on block 0.
                    pas, pbs = {}, {}
                    for b in blocks:
                        pas[b] = psum.tile([P, P], F32, tag="pa",
                                           bufs=SB, space="PSUM",
                                           name=f"pa{b}")
                        pbs[b] = psum.tile([P, P], F32, tag="pb",
                                           bufs=SB, space="PSUM",
                                           name=f"pb{b}")
                    nz = {b: [g for g in range(G) if c_bg[b][g] > 0]
                          for b in blocks}
                    for g in range(G):
                        for b in blocks:
                            if g not in nz[b]:
                                continue
                            c0 = lay.sb_cols[si][(g, b)]
                            for j in range(int(c_bg[b][g])):
                                c = c0 + j
                                m = m_tile(gc_base + c, on_act=(c % 10) >= 7)
                                st = g == nz[b][0] and j == 0
                                sp = (g == nz[b][-1]
                                      and j == c_bg[b][g] - 1)
                                nc.tensor.matmul(pas[b][:],
                                                 lhsT=msg[:, c, 0:P],
                                                 rhs=m[:], start=st, stop=sp)
                                nc.tensor.matmul(pbs[b][:],
                                                 lhsT=msg[:, c, P:F_h],
                                                 rhs=m[:], start=st, stop=sp)
                    for b in blocks:
                        ra = work.tile([P, P], F32, tag="ra", bufs=2)
                        rb = work.tile([P, P], F32, tag="rb", bufs=2)
                        nc.scalar.activation(ra[:], pas[b][:], Relu,
                                             bias=b1_t[:, 0:1], scale=1.0)
                        nc.scalar.activation(rb[:], pbs[b][:], Relu,
                                             bias=b1_t[:, 1:2], scale=1.0)
                        pg = psum.tile([P, F_out], F32, tag="pgo", bufs=2,
                                       space="PSUM")
                        nc.tensor.matmul(pg[:], lhsT=ra[:], rhs=w2_t[:, 0, :],
                                         start=True, stop=False)
                        nc.tensor.matmul(pg[:], lhsT=rb[:], rhs=w2_t[:, 1, :],
                                         start=False, stop=True)
                        g_sb = work.tile([P, F_out], BF16, tag="gsb", bufs=3)
                        nc.scalar.activation(g_sb[:], pg[:], Copy, scale=1.0)
                        bw = b - grp[0]
                        nc.sync.dma_start(g_own[gi][bw * P:(bw + 1) * P, :],
                                          g_sb[:])
                    si += 1
                nc.gpsimd.collective_compute(
                    "AllGather", mybir.AluOpType.bypass, replica_groups=rg,
                    ins=[g_own[gi][:]], outs=[g_full[gi][:]],
                )

            # ---- phase E: aggregate layer 2, add bias, write out ----
            # Pre-emit the first msg-buffer-depth superblocks' gathers for
            # groups 0..G-2 before any last-group op: the last-group gathers
            # embed a wait on the final AG2 chunk, and the gpsimd FIFO is
            # strictly in-order, so emitting them last lets the early-group
            # traffic flow during phase C's tail instead of stalling behind
            # that wait.
            pre_msgs = {}
            if G > 1:
                for si0 in range(min(3, len(lay.sbs))):
                    wp = lay.sb_width(si0)
                    pm = work.tile([P, wp, F_out], BF16, tag="msg", bufs=3,
                                   name=f"emsg{si0}")
                    gathers(pm, si0, g_full, F_out, only_g=range(G - 1))
                    pre_msgs[si0] = pm
            si = 0
            for gi, grp in enumerate(groups):
                for s0 in range(0, len(grp), SB):
                    blocks = lay.sbs[si]
                    w = lay.sb_width(si)
                    gc_base = lay.sb_base[si]
                    if si in pre_msgs:
                        msg2 = pre_msgs[si]
                        gathers(msg2, si, g_full, F_out, only_g=[G - 1])
                    else:
                        msg2 = work.tile([P, w, F_out], BF16, tag="msg",
                                         bufs=3)
                        gathers(msg2, si, g_full, F_out)
                    pos_ = {}
                    for b in blocks:
                        pos_[b] = psum.tile([P, F_out], F32, tag="pa",
                                            bufs=SB, space="PSUM",
                                            name=f"po{b}")
                    nz = {b: [g for g in range(G) if c_bg[b][g] > 0]
                          for b in blocks}
                    for g in range(G):
                        for b in blocks:
                            if g not in nz[b]:
                                continue
                            c0 = lay.sb_cols[si][(g, b)]
                            for j in range(int(c_bg[b][g])):
                                c = c0 + j
                                m = m_tile(gc_base + c, on_act=(c % 10) >= 7)
                                st = g == nz[b][0] and j == 0
                                sp = (g == nz[b][-1]
                                      and j == c_bg[b][g] - 1)
                                nc.tensor.matmul(pos_[b][:], lhsT=m[:],
                                                 rhs=msg2[:, c, :],
                                                 start=st, stop=sp)
                    for b in blocks:
                        o_sb = work.tile([P, F_out], F32, tag="osb", bufs=3)
                        nc.vector.tensor_tensor(out=o_sb[:], in0=pos_[b][:],
                                                in1=b2_t[:],
                                                op=mybir.AluOpType.add)
                        nc.sync.dma_start(out_d[b * P:(b + 1) * P, :], o_sb[:])
                    si += 1
            psumC.__exit__(None, None, None)

    nc.compile()
    return nc


def _in_maps(cfg, cores, shared):
    return [{**shared, **c} for c in cores]


def _assemble(cfg, outs):
    N, F_out, npc_raw = cfg["N"], cfg["F_out"], cfg["npc_raw"]
    pos = cfg["pos"]
    full = np.empty((N, F_out), np.float32)
    for c in range(NCORES):
        nodes = np.arange(c * npc_raw, (c + 1) * npc_raw)
        full[nodes] = outs[c][pos[nodes]]
    return full


# ---------------------------------------------------------------------------
# entry points
# ---------------------------------------------------------------------------

def kernel(x, edge_index, W1, b1, W2, b2):
    cfg, cores, shared = _prep(x, edge_index, W1, b1, W2, b2)
    nc = _build_nc(cfg)
    from concourse.bass_utils import run_bass_kernel_spmd
    res = run_bass_kernel_spmd(nc, _in_maps(cfg, cores, shared),
                               list(range(NCORES)))
    return _assemble(cfg, [r["out"] for r in res.results])


def run_profiled(x, edge_index, W1, b1, W2, b2, tmpdir=None):
    """Like kernel(), but traces on HW; returns (out, exec_time_ns, tmpdir)."""
    import time

    t0 = time.time()
    cfg, cores, shared = _prep(x, edge_index, W1, b1, W2, b2)
    print(f"prep {time.time() - t0:.1f}s; chunks/phase={int(cfg['c_bg'].sum())} "
          f"nb={cfg['nb']}")
    t0 = time.time()
    nc = _build_nc(cfg)
    print(f"build {time.time() - t0:.1f}s; {len(nc.inst_map)} instructions")
    from concourse.bass_utils import run_bass_kernel_spmd
    in_maps = _in_maps(cfg, cores, shared)
    t0 = time.time()
    res = run_bass_kernel_spmd(nc, in_maps, list(range(NCORES)))
    print(f"run {time.time() - t0:.1f}s")
    out = _assemble(cfg, [r["out"] for r in res.results])
    exec_ns = None
    try:
        t0 = time.time()
        res2 = run_bass_kernel_spmd(nc, in_maps, list(range(NCORES)),
                                    trace=True, tmpdir=tmpdir)
        print(f"traced run {time.time() - t0:.1f}s")
        exec_ns = res2.exec_time_ns
    except Exception as e:
        print(f"trace run failed: {type(e).__name__}: {str(e)[:200]}")
    return out, exec_ns, tmpdir


def _numpy_ref(x, edge_index, W1, b1, W2, b2):
    N = x.shape[0]
    src = np.concatenate([edge_index[0], np.arange(N)])
    dst = np.concatenate([edge_index[1], np.arange(N)])
    deg = np.bincount(dst, minlength=N).astype(np.float64)
    dinv = np.where(deg > 0, 1 / np.sqrt(deg), 0)
    nrm = (dinv[src] * dinv[dst]).astype(np.float32)

    def layer(h, W, b):
        hw = h @ W
        out = np.zeros((N, W.shape[1]), np.float32)
        np.add.at(out, dst, hw[src] * nrm[:, None])
        return out + b

    h = np.maximum(layer(x, W1, b1), 0)
    return layer(h, W2, b2)


def _selftest_sim():
    from concourse import bass_interp
    rng = np.random.default_rng(1)
    N, E, F_in = 2048, 8192, 512
    x = rng.standard_normal((N, F_in), dtype=np.float32)
    ei = rng.integers(0, N, (2, E)).astype(np.int64)
    W1 = (rng.standard_normal((F_in, 256), dtype=np.float32) * F_in ** -0.5)
    W2 = (rng.standard_normal((256, 128), dtype=np.float32) * 256 ** -0.5)
    b1 = rng.standard_normal(256).astype(np.float32) * 0.1
    b2 = rng.standard_normal(128).astype(np.float32) * 0.1

    cfg, cores, shared = _prep(x, ei, W1, b1, W2, b2)
    print("cfg:", {k: (v if k != "c_bg" else v.tolist())
                   for k, v in cfg.items() if k != "pos"})
    nc = _build_nc(cfg)
    print("built; instructions:", len(nc.inst_map))

    sim = bass_interp.MultiCoreSim(nc, NCORES)
    for i, m in enumerate(_in_maps(cfg, cores, shared)):
        for k, v in m.items():
            sim.cores[i].tensor(k)[:] = v
    sim.simulate()
    outs = [np.array(sim.cores[i].mem_tensor("out")) for i in range(NCORES)]
    got = _assemble(cfg, outs)
    want = _numpy_ref(x, ei, W1, b1, W2, b2)
    err = np.abs(got - want).max() / (np.abs(want).max() + 1e-9)
    print("selftest rel err:", err)
    assert err < 1e-2, "selftest FAILED"
    print("SELFTEST PASSED")


if __name__ == "__main__":
    _selftest_sim()


# revision 49
# speedup vs baseline: 1.0051x; 1.0051x over previous
"""2-layer GCN (GCNConv x2, symmetric norm, self-loops) on 8 Trainium2 NeuronCores.

Strategy (graph/data parallel) — chunked collectives + ragged chunks:
  - Nodes are partitioned contiguously across 8 cores (6250/core, padded to
    6272 = 49*128). Within a core, nodes are permuted into 49 blocks of 128
    with greedy in-degree balancing (two rounds, since an edge's src group
    depends on the src node's block assignment).
  - Blocks are split into G=2 groups. The gather tables are per-group Shared
    DRAM tiles (h_full_g / g_full_g, <= 25600 rows so dma_gather's int16
    indices cover them); each AllGather is a per-group collective issued as
    soon as that group's rows are computed, overlapping remaining compute.
  - Edge slots are grouped by (dst block, src group) with RAGGED chunk
    counts: chunk count for (b, g) is the max over the 8 cores of that
    bucket's edge count (a compile-time constant, so the SPMD program is
    shared), not a global max — measured 911 chunks/core/phase vs 980 for
    a global split (7% less gather traffic / M work / matmuls).
  - Layer 1 dense transform h = x @ W1 is row-sharded: each core multiplies
    its [6272, 4096] x-slice (fed pre-transposed, bf16) against replicated
    W1; AG1 chunk g is issued right after group g's blocks are stored.
  - Aggregation: dma_gather fetches h[src] rows per edge slot; a one-hot
    selection matrix M scatter-adds them on the TensorEngine with PSUM
    accumulation. norm = dinv[src]*dinv[dst] is folded into M, so padded
    slots (norm=0) are inert. M tiles are built ~70% on DVE (tensor_scalar
    is_equal*mult) and ~30% on ACT (|mdst-iota| then relu(norm - norm*t),
    exact for the integer grid) — both engines run ~2ns/col, so splitting
    the ~1 op / 128 edges cost is what keeps M construction off the
    critical path.
  - bias+ReLU and the layer-2 transform @ W2 run per 128-node block (PSUM ->
    SBUF copies on the ACT engine); AG2 chunk g is issued after group g's
    blocks finish, then the same gather/scatter aggregates layer 2. Matmuls
    run in two passes per superblock (every block's g0 chunks, then g1) so
    the PE is not head-of-line blocked while an AG chunk is still landing.

kernel(**inputs) takes full unsharded inputs, returns the full [50000, 128]
output. Self-contained: no sibling imports; /opt/trn_rl_repo provides bass.
"""

import math
import sys

import numpy as np

sys.path.insert(0, "/opt/trn_rl_repo")

import concourse.bass as bass  # noqa: E402
import concourse.mybir as mybir  # noqa: E402
import concourse.tile as tile  # noqa: E402
from concourse import bacc  # noqa: E402

P = 128
NCORES = 8
SB = 3  # blocks per gather superblock
GMAX = 6  # chunks (of 128 idxs) per dma_gather op; 6*128=768 is the HW cap

F32 = mybir.dt.float32
BF16 = mybir.dt.bfloat16
I16 = mybir.dt.int16


# ---------------------------------------------------------------------------
# host-side preprocessing
# ---------------------------------------------------------------------------

def _groups(nb):
    """Block groups for chunked AllGathers (G=2: best ragged-padding vs
    collective-overlap tradeoff; more groups round more partial chunks)."""
    g = min(2, nb)
    gb = math.ceil(nb / g)
    return [list(range(s, min(s + gb, nb))) for s in range(0, nb, gb)]


def _superblocks(groups):
    sbs = []
    for g in groups:
        for s in range(0, len(g), SB):
            sbs.append(tuple(g[s:s + SB]))
    return sbs


def _swizzle_idx(idx):
    """gather idx j -> [j%16, j//16], replicated across the 8 groups of 16."""
    n = idx.shape[0]
    a = np.zeros((16, n // 16), np.int16)
    a[np.arange(n) % 16, np.arange(n) // 16] = idx.astype(np.int16)
    return np.tile(a, (8, 1))


def _swizzle_slot(v):
    """slot s -> [s%128, s//128] (dma_gather / matmul-chunk layout)."""
    n = v.shape[0]
    a = np.zeros((P, n // P), np.float32)
    a[np.arange(n) % P, np.arange(n) // P] = v.astype(np.float32)
    return a


def _pack_blocks(loads, nb):
    """Greedy bin-pack node ids (by descending load) into nb blocks of <=128.

    Returns pos[i] = block*128 + slot for each local node i."""
    import heapq

    n = loads.shape[0]
    order = np.argsort(-loads, kind="stable")
    heap = [(0.0, b) for b in range(nb)]
    heapq.heapify(heap)
    counts = np.zeros(nb, np.int64)
    pos = np.empty(n, np.int64)
    for i in order:
        while True:
            load, b = heapq.heappop(heap)
            if counts[b] < P:
                break
        pos[i] = b * P + counts[b]
        counts[b] += 1
        if counts[b] < P:
            heapq.heappush(heap, (load + loads[i], b))
    return pos


def _pack_blocks_nd(d, nb):
    """Greedy pack minimizing the per-block max over src-group loads."""
    n, G = d.shape
    tot = d.sum(axis=1)
    order = np.argsort(-tot, kind="stable")
    cur = np.zeros((nb, G))
    counts = np.zeros(nb, np.int64)
    pos = np.empty(n, np.int64)
    for i in order:
        nxt = cur + d[i]
        score = nxt.max(axis=1) + 0.001 * nxt.sum(axis=1)
        score[counts >= P] = np.inf
        b = int(np.argmin(score))
        pos[i] = b * P + counts[b]
        counts[b] += 1
        cur[b] += d[i]
    return pos


class Layout:
    """Static ragged chunk layout shared by host prep and device build.

    c_bg[b][g]: chunks for (dst block b, src group g) = max edge count over
    cores, rounded up to 128s. All derived offsets are compile-time."""

    def __init__(self, nb, groups, c_bg):
        self.nb = nb
        self.groups = groups
        self.G = len(groups)
        self.c_bg = c_bg
        self.sbs = _superblocks(groups)
        # idx table: for g: for b: c_bg[b][g] chunks (contiguous per group)
        self.idx_off = np.zeros((self.G, nb + 1), np.int64)
        off = 0
        for g in range(self.G):
            for b in range(nb):
                self.idx_off[g][b] = off
                off += c_bg[b][g]
            self.idx_off[g][nb] = off
        self.total_chunks = off
        # msg/m-data chunk order: per superblock: for g: for b in sb: chunks
        self.sb_base = {}
        self.sb_cols = {}   # (sb_index) -> dict[(g, b)] = col offset in sb
        base = 0
        for si, blocks in enumerate(self.sbs):
            cols = {}
            c = 0
            for g in range(self.G):
                for b in blocks:
                    cols[(g, b)] = c
                    c += c_bg[b][g]
            self.sb_base[si] = base
            self.sb_cols[si] = cols
            self.sb_len = None
            base += c
        self.total_mchunks = base

    def sb_width(self, si):
        blocks = self.sbs[si]
        return sum(self.c_bg[b][g] for b in blocks for g in range(self.G))

    def block_cols(self, si, b):
        """msg column indices for block b's chunks, group-ascending."""
        cols = []
        for g in range(self.G):
            c0 = self.sb_cols[si][(g, b)]
            cols.extend(range(c0, c0 + self.c_bg[b][g]))
        return cols


def _prep(x, edge_index, W1, b1, W2, b2):
    N, F_in = x.shape
    F_h = W1.shape[1]
    F_out = W2.shape[1]
    assert N % NCORES == 0 and F_in % P == 0 and F_h == 2 * P and F_out == P
    npc_raw = N // NCORES
    nb = math.ceil(npc_raw / P)
    npc = nb * P

    groups = _groups(nb)
    G = len(groups)
    gb = len(groups[0])
    glen = np.array([len(g) for g in groups])
    assert int(glen.max()) * P * NCORES < 2 ** 15
    grp_of_blk = np.arange(nb) // gb
    first_blk = np.array([g[0] for g in groups])

    src = np.concatenate([np.asarray(edge_index[0]), np.arange(N)]).astype(np.int64)
    dst = np.concatenate([np.asarray(edge_index[1]), np.arange(N)]).astype(np.int64)
    deg = np.bincount(dst, minlength=N).astype(np.float64)
    dinv = np.where(deg > 0, 1.0 / np.sqrt(deg), 0.0)
    norm = (dinv[src] * dinv[dst]).astype(np.float32)

    core_dst = dst // npc_raw

    # --- round 1: pack by total in-degree to get provisional src groups ---
    pos1 = np.empty(N, np.int64)
    for c in range(NCORES):
        nodes = np.arange(c * npc_raw, (c + 1) * npc_raw)
        pos1[nodes] = _pack_blocks(deg[nodes], nb)
    g1 = grp_of_blk[pos1[src] // P]

    # --- round 2: pack balancing per-src-group in-degree ---
    d_by_g = np.zeros((N, G))
    np.add.at(d_by_g, (dst, g1), 1)
    pos = np.empty(N, np.int64)
    for c in range(NCORES):
        nodes = np.arange(c * npc_raw, (c + 1) * npc_raw)
        pos[nodes] = _pack_blocks_nd(d_by_g[nodes], nb)

    # final groups and in-group rows from the round-2 packing. Group 0's
    # table gets one dummy block per core: its dummy rows in h_own_0 are
    # written at the END of phase A so AG1_0 cannot start early and steal
    # HBM bandwidth from the x@W1 stream (which HAM-rethrottles the PE);
    # g_own_0's dummy rows are written at the START of phase C so AG2_0
    # still fires as early as possible.
    pad_blk = np.zeros(G, np.int64)
    if G > 1:
        pad_blk[0] = 1
    assert int((glen + pad_blk).max()) * P * NCORES < 2 ** 15
    blk = pos // P
    slot = pos % P
    grp = grp_of_blk[blk]
    core_of = np.arange(N) // npc_raw
    row_in_grp = (core_of * (glen + pad_blk)[grp]
                  + (blk - first_blk[grp])) * P + slot
    e_grp = grp[src]
    e_row = row_in_grp[src]

    # per (core, block, group) edge counts -> ragged chunk constants
    blk_of_dst = blk[dst]
    cnt = np.zeros((NCORES, nb, G), np.int64)
    np.add.at(cnt, (core_dst, blk_of_dst, e_grp), 1)
    c_bg = np.ceil(cnt.max(axis=0) / P).astype(np.int64)  # [nb, G]
    lay = Layout(nb, groups, c_bg)

    cores = []
    for c in range(NCORES):
        mask = core_dst == c
        e_src_row = e_row[mask]
        e_g = e_grp[mask]
        e_blk = blk_of_dst[mask]
        e_dl = (pos[dst[mask]] % P).astype(np.float32)
        e_nrm = norm[mask]

        idx_flat = np.zeros(lay.total_chunks * P, np.int64)
        dl_by_chunk = {}   # (g, b) -> [c_bg, P]
        nrm_by_chunk = {}
        for g in range(G):
            sel = e_g == g
            sr = e_src_row[sel]
            bl = e_blk[sel]
            order = np.argsort(bl, kind="stable")
            sr, bl = sr[order], bl[order]
            dl_s, nr_s = e_dl[sel][order], e_nrm[sel][order]
            start = np.searchsorted(bl, np.arange(nb))
            end = np.searchsorted(bl, np.arange(nb) + 1)
            for b in range(nb):
                c_g = int(c_bg[b][g])
                k = end[b] - start[b]
                assert k <= c_g * P, (k, c_g)
                sl = slice(start[b], end[b])
                o0 = lay.idx_off[g][b] * P
                idx_flat[o0: o0 + k] = sr[sl]
                flat_dl = np.zeros(c_g * P, np.float32)
                flat_nr = np.zeros(c_g * P, np.float32)
                flat_dl[:k] = dl_s[sl]
                flat_nr[:k] = nr_s[sl]
                dl_by_chunk[(g, b)] = flat_dl.reshape(c_g, P)
                nrm_by_chunk[(g, b)] = flat_nr.reshape(c_g, P)
            seg = idx_flat[lay.idx_off[g][0] * P: lay.idx_off[g][nb] * P]
            assert seg.min() >= 0
            assert seg.max(initial=0) < (glen[g] + pad_blk[g]) * P * NCORES

        # mdata in device chunk order: per superblock: for g: for b: chunks
        md, mn = [], []
        for si, blocks in enumerate(lay.sbs):
            for g in range(G):
                for b in blocks:
                    md.append(dl_by_chunk[(g, b)])
                    mn.append(nrm_by_chunk[(g, b)])
        mdst = np.concatenate(md).reshape(lay.total_mchunks * P)
        mnorm = np.concatenate(mn).reshape(lay.total_mchunks * P)

        # x slice, permuted and transposed: xt[f, pos] = x[node, f]
        nodes = np.arange(c * npc_raw, (c + 1) * npc_raw)
        xp = np.zeros((npc, F_in), np.float32)
        xp[pos[nodes]] = np.asarray(x[nodes], np.float32)
        xt = np.ascontiguousarray(xp.T.astype(np.float32))
        import ml_dtypes
        xt = xt.astype(ml_dtypes.bfloat16)

        cores.append({
            "xt": xt,
            "idxs": _swizzle_idx(idx_flat),
            "mdst": _swizzle_slot(mdst),
            "mnorm": _swizzle_slot(mnorm),
            "mnormneg": _swizzle_slot(-mnorm),
        })

    import ml_dtypes
    iota = np.tile(np.arange(P, dtype=np.float32)[None, :], (P, 1))
    shared = {
        "w1": ml_dtypes.bfloat16(np.asarray(W1, np.float32)),
        "w2": np.asarray(W2, np.float32),
        "b1p": np.asarray(b1, np.float32).reshape(2, P).T.copy(),
        "b2b": np.tile(np.asarray(b2, np.float32)[None, :], (P, 1)),
        "iota": iota.astype(ml_dtypes.bfloat16),
    }
    cfg = dict(N=N, F_in=F_in, F_h=F_h, F_out=F_out, npc_raw=npc_raw, nb=nb,
               npc=npc, c_bg=c_bg, pos=pos)
    return cfg, cores, shared


# ---------------------------------------------------------------------------
# device kernel
# ---------------------------------------------------------------------------

def _build_nc(cfg):
    F_in, F_h, F_out = cfg["F_in"], cfg["F_h"], cfg["F_out"]
    nb, npc = cfg["nb"], cfg["npc"]
    c_bg = cfg["c_bg"]
    kt = F_in // P
    groups = _groups(nb)
    G = len(groups)
    lay = Layout(nb, groups, c_bg)
    nchunks = lay.total_mchunks
    rg = [list(range(NCORES))]

    nc = bacc.Bacc(None, num_devices=NCORES, num_swdge_queues=4)

    xt_d = nc.declare_dram_parameter("xt", [F_in, npc], BF16, isOutput=False)
    w1_d = nc.declare_dram_parameter("w1", [F_in, F_h], BF16, isOutput=False)
    w2_d = nc.declare_dram_parameter("w2", [F_h, F_out], F32, isOutput=False)
    b1_d = nc.declare_dram_parameter("b1p", [P, 2], F32, isOutput=False)
    b2_d = nc.declare_dram_parameter("b2b", [P, F_out], F32, isOutput=False)
    iota_d = nc.declare_dram_parameter("iota", [P, P], BF16, isOutput=False)
    idxs_d = nc.declare_dram_parameter("idxs", [P, lay.total_chunks * 8], I16,
                                       isOutput=False)
    mdst_d = nc.declare_dram_parameter("mdst", [P, nchunks], F32, isOutput=False)
    mnorm_d = nc.declare_dram_parameter("mnorm", [P, nchunks], F32, isOutput=False)
    mneg_d = nc.declare_dram_parameter("mnormneg", [P, nchunks], F32,
                                       isOutput=False)
    out_d = nc.declare_dram_parameter("out", [npc, F_out], F32, isOutput=True)

    with tile.TileContext(nc) as tc:
        with (
            tc.tile_pool(name="const", bufs=1) as const,
            tc.tile_pool(name="work", bufs=1) as work,
            tc.tile_pool(name="dram", bufs=1, space="DRAM") as dram,
        ):
            # Group 0 gets one dummy block per core (see _prep): its h rows
            # are written at the end of phase A to postpone AG1_0.
            pad = [1 if (i == 0 and G > 1) else 0 for i in range(G)]
            h_own = [dram.tile([(len(g) + pad[i]) * P, F_h], BF16,
                               name=f"h_own{i}")
                     for i, g in enumerate(groups)]
            h_full = [dram.tile([(len(g) + pad[i]) * P * NCORES, F_h], BF16,
                                addr_space="Shared", name=f"h_full{i}")
                      for i, g in enumerate(groups)]
            g_own = [dram.tile([(len(g) + pad[i]) * P, F_out], BF16,
                               name=f"g_own{i}")
                     for i, g in enumerate(groups)]
            g_full = [dram.tile([(len(g) + pad[i]) * P * NCORES, F_out], BF16,
                                addr_space="Shared", name=f"g_full{i}")
                      for i, g in enumerate(groups)]

            w1_t = const.tile([P, kt, F_h], BF16)
            w2_t = const.tile([P, 2, F_out], F32)
            b1_t = const.tile([P, 2], F32)
            b2_t = const.tile([P, F_out], F32)
            iota_t = const.tile([P, P], BF16)
            idxs_t = const.tile([P, lay.total_chunks * 8], I16)
            mdst_t = const.tile([P, nchunks], F32)
            mnorm_t = const.tile([P, nchunks], F32)
            mneg_t = const.tile([P, nchunks], F32)

            nc.sync.dma_start(w1_t[:], w1_d[:].rearrange("(a p) o -> p a o", p=P))
            nc.sync.dma_start(w2_t[:], w2_d[:].rearrange("(h p) o -> p h o", p=P))
            nc.sync.dma_start(b1_t[:], b1_d[:])
            nc.sync.dma_start(b2_t[:], b2_d[:])
            nc.sync.dma_start(iota_t[:], iota_d[:])
            nc.sync.dma_start(idxs_t[:], idxs_d[:])
            nc.sync.dma_start(mdst_t[:], mdst_d[:])
            nc.sync.dma_start(mnorm_t[:], mnorm_d[:])
            nc.sync.dma_start(mneg_t[:], mneg_d[:])

            # ---- phase A: h = x @ W1, AG1 chunk per block group ----
            xt_r = xt_d[:].rearrange("(a p) n -> p a n", p=P)
            # Sub-groups of 8 blocks: xt DMAs read 8*128 contiguous columns
            # (2 KB/partition descriptors, ~300 GB/s); PSUM holds two blocks
            # per bank ([P, 2, F_h] f32 = one 2 KB bank), 4 banks per
            # sub-group, bufs=8 = all 8 banks double-buffered.
            GA = 8
            psumA = tc.tile_pool(name="psumA", bufs=1, space="PSUM")
            psum = psumA.__enter__()
            dummy_done = False
            for gi, grp in enumerate(groups):
                for s0 in range(0, len(grp), GA):
                    gblk = grp[s0:s0 + GA]
                    g0 = gblk[0]
                    phs = [psum.tile([P, F_h], F32, tag="ph", bufs=8,
                                     space="PSUM", name=f"ph{g0}_{i}")
                           for i in range(len(gblk))]
                    for a in range(kt):
                        xt_t = work.tile([P, len(gblk) * P], BF16, tag="xt",
                                         bufs=16)
                        nc.sync.dma_start(
                            xt_t[:], xt_r[:, a, g0 * P:g0 * P + len(gblk) * P])
                        for i in range(len(gblk)):
                            nc.tensor.matmul(phs[i][:],
                                             lhsT=xt_t[:, i * P:(i + 1) * P],
                                             rhs=w1_t[:, a, :],
                                             start=(a == 0), stop=(a == kt - 1))
                    for i, b in enumerate(gblk):
                        h_sb = work.tile([P, F_h], BF16, tag="hsb", bufs=3)
                        nc.vector.tensor_copy(h_sb[:], phs[i][:])
                        bw = b - grp[0]
                        nc.sync.dma_start(h_own[gi][bw * P:(bw + 1) * P, :],
                                          h_sb[:])
                    if (pad[0] and not dummy_done and gi == G - 1
                            and s0 + 3 * GA >= len(grp)):
                        # dummy block write (content irrelevant, never
                        # gathered): sequenced after ~66% of phase A's h
                        # writes so AG1_0 runs during A's last stretch and
                        # completes right around A's end, without starving
                        # the xt stream for most of A.
                        d0 = len(groups[0]) * P
                        nc.sync.dma_start(h_own[0][d0:d0 + P, :],
                                          w1_t[:, 0, :])
                        dummy_done = True
            for gi in range(G):
                nc.gpsimd.collective_compute(
                    "AllGather", mybir.AluOpType.bypass, replica_groups=rg,
                    ins=[h_own[gi][:]], outs=[h_full[gi][:]],
                )

            psumA.__exit__(None, None, None)
            psumC = tc.tile_pool(name="psumC", bufs=1, space="PSUM")
            psum = psumC.__enter__()

            qn = [0]

            def gathers(dst_t, si, tables, elem, only_g=None):
                """Per-src-group gathers for superblock si (<=GMAX chunks/op
                to stay under the 768-idx dma_gather HW cap; round-robin the
                4 SWDGE queues).

                (prepare_only/trigger_dma pipelining was tried to hide the
                ~1us per-op reissue bubble but crashes the NRT runtime on
                this stack; keep the direct path.)"""
                blocks = lay.sbs[si]
                for g in range(G):
                    if only_g is not None and g not in only_g:
                        continue
                    nch = sum(int(c_bg[b][g]) for b in blocks)
                    if nch == 0:
                        continue
                    c0 = lay.sb_cols[si][(g, blocks[0])]
                    i0 = int(lay.idx_off[g][blocks[0]])
                    for s in range(0, nch, GMAX):
                        kk = min(GMAX, nch - s)
                        nc.gpsimd.dma_gather(
                            out_ap=dst_t[:, c0 + s:c0 + s + kk, :],
                            in_ap=tables[g][:],
                            idxs_ap=idxs_t[:, (i0 + s) * 8:(i0 + s + kk) * 8],
                            num_idxs=kk * P, num_idxs_reg=kk * P,
                            elem_size=elem, queue_num=qn[0] % 4)
                        qn[0] += 1

            Abs = mybir.ActivationFunctionType.Abs
            Copy = mybir.ActivationFunctionType.Copy
            Relu = mybir.ActivationFunctionType.Relu

            def m_tile(gc, on_act=False):
                """M[s, d] = (dst[s] == d) * norm[s].

                DVE path: one is_equal*mult tensor_scalar. ACT path (~30% of
                tiles, splitting the ~2ns/col cost across both engines):
                t = |mdst - iota|; m = relu(-norm*t + norm) == one-hot*norm
                exactly on the integer grid."""
                m = work.tile([P, P], BF16, tag="m", bufs=128)
                if on_act:
                    t = work.tile([P, P], BF16, tag="mt", bufs=16)
                    nc.scalar.activation(t[:], iota_t[:], Abs,
                                         bias=mdst_t[:, gc:gc + 1], scale=-1.0)
                    nc.scalar.activation(m[:], t[:], Relu,
                                         bias=mnorm_t[:, gc:gc + 1],
                                         scale=mneg_t[:, gc:gc + 1])
                else:
                    nc.vector.tensor_scalar(
                        out=m[:], in0=iota_t[:],
                        scalar1=mdst_t[:, gc:gc + 1],
                        scalar2=mnorm_t[:, gc:gc + 1],
                        op0=mybir.AluOpType.is_equal, op1=mybir.AluOpType.mult)
                return m

            # ---- phase C: aggregate layer 1, relu, transform by W2 ----
            if pad[0]:
                # g_own_0's dummy block, written up front so AG2_0 fires as
                # soon as group 0's real blocks are done.
                d0 = len(groups[0]) * P
                nc.sync.dma_start(g_own[0][d0:d0 + P, :], iota_t[:])
            # Same FIFO-ordering trick as phase E below: pre-emit the
            # first superblocks' early-group gathers so they are not stuck
            # behind a last-group op waiting on the final AG1 chunk.
            pre_cmsgs = {}
            if G > 1:
                for si0 in range(min(3, len(lay.sbs))):
                    wp = lay.sb_width(si0)
                    pc = work.tile([P, wp, F_h], BF16, tag="msg", bufs=3,
                                   name=f"cmsg{si0}")
                    gathers(pc, si0, h_full, F_h, only_g=range(G - 1))
                    pre_cmsgs[si0] = pc
            si = 0
            for gi, grp in enumerate(groups):
                for s0 in range(0, len(grp), SB):
                    blocks = lay.sbs[si]
                    w = lay.sb_width(si)
                    gc_base = lay.sb_base[si]
                    if si in pre_cmsgs:
                        msg = pre_cmsgs[si]
                        gathers(msg, si, h_full, F_h, only_g=[G - 1])
                    else:
                        msg = work.tile([P, w, F_h], BF16, tag="msg", bufs=3)
                        gathers(msg, si, h_full, F_h)
                    # Two passes (all blocks' g0 chunks, then g1): during
                    # the AG-chunk-g1 wait the PE drains every block's g0
                    # work instead of head-of-line blocking on block 0.
                    pas, pbs = {}, {}
                    for b in blocks:
                        pas[b] = psum.tile([P, P], F32, tag="pa",
                                           bufs=SB, space="PSUM",
                                           name=f"pa{b}")
                        pbs[b] = psum.tile([P, P], F32, tag="pb",
                                           bufs=SB, space="PSUM",
                                           name=f"pb{b}")
                    nz = {b: [g for g in range(G) if c_bg[b][g] > 0]
                          for b in blocks}
                    for g in range(G):
                        for b in blocks:
                            if g not in nz[b]:
                                continue
                            c0 = lay.sb_cols[si][(g, b)]
                            for j in range(int(c_bg[b][g])):
                                c = c0 + j
                                m = m_tile(gc_base + c, on_act=(c % 10) >= 7)
                                st = g == nz[b][0] and j == 0
                                sp = (g == nz[b][-1]
                                      and j == c_bg[b][g] - 1)
                                nc.tensor.matmul(pas[b][:],
                                                 lhsT=msg[:, c, 0:P],
                                                 rhs=m[:], start=st, stop=sp)
                                nc.tensor.matmul(pbs[b][:],
                                                 lhsT=msg[:, c, P:F_h],
                                                 rhs=m[:], start=st, stop=sp)
                    for b in blocks:
                        ra = work.tile([P, P], F32, tag="ra", bufs=2)
                        rb = work.tile([P, P], F32, tag="rb", bufs=2)
                        nc.scalar.activation(ra[:], pas[b][:], Relu,
                                             bias=b1_t[:, 0:1], scale=1.0)
                        nc.scalar.activation(rb[:], pbs[b][:], Relu,
                                             bias=b1_t[:, 1:2], scale=1.0)
                        pg = psum.tile([P, F_out], F32, tag="pgo", bufs=2,
                                       space="PSUM")
                        nc.tensor.matmul(pg[:], lhsT=ra[:], rhs=w2_t[:, 0, :],
                                         start=True, stop=False)
                        nc.tensor.matmul(pg[:], lhsT=rb[:], rhs=w2_t[:, 1, :],
                                         start=False, stop=True)
                        g_sb = work.tile([P, F_out], BF16, tag="gsb", bufs=3)
                        nc.scalar.activation(g_sb[:], pg[:], Copy, scale=1.0)
                        bw = b - grp[0]
                        nc.sync.dma_start(g_own[gi][bw * P:(bw + 1) * P, :],
                                          g_sb[:])
                    si += 1
                nc.gpsimd.collective_compute(
                    "AllGather", mybir.AluOpType.bypass, replica_groups=rg,
                    ins=[g_own[gi][:]], outs=[g_full[gi][:]],
                )

            # ---- phase E: aggregate layer 2, add bias, write out ----
            # Pre-emit the first msg-buffer-depth superblocks' gathers for
            # groups 0..G-2 before any last-group op: the last-group gathers
            # embed a wait on the final AG2 chunk, and the gpsimd FIFO is
            # strictly in-order, so emitting them last lets the early-group
            # traffic flow during phase C's tail instead of stalling behind
            # that wait.
            pre_msgs = {}
            if G > 1:
                for si0 in range(min(3, len(lay.sbs))):
                    wp = lay.sb_width(si0)
                    pm = work.tile([P, wp, F_out], BF16, tag="msg", bufs=3,
                                   name=f"emsg{si0}")
                    gathers(pm, si0, g_full, F_out, only_g=range(G - 1))
                    pre_msgs[si0] = pm
            si = 0
            for gi, grp in enumerate(groups):
                for s0 in range(0, len(grp), SB):
                    blocks = lay.sbs[si]
                    w = lay.sb_width(si)
                    gc_base = lay.sb_base[si]
                    if si in pre_msgs:
                        msg2 = pre_msgs[si]
                        gathers(msg2, si, g_full, F_out, only_g=[G - 1])
                    else:
                        msg2 = work.tile([P, w, F_out], BF16, tag="msg",
                                         bufs=3)
                        gathers(msg2, si, g_full, F_out)
                    pos_ = {}
                    for b in blocks:
                        pos_[b] = psum.tile([P, F_out], F32, tag="pa",
                                            bufs=SB, space="PSUM",
                                            name=f"po{b}")
                    nz = {b: [g for g in range(G) if c_bg[b][g] > 0]
                          for b in blocks}
                    for g in range(G):
                        for b in blocks:
                            if g not in nz[b]:
                                continue
                            c0 = lay.sb_cols[si][(g, b)]
                            for j in range(int(c_bg[b][g])):
                                c = c0 + j
                                m = m_tile(gc_base + c, on_act=(c % 10) >= 7)
                                st = g == nz[b][0] and j == 0
                                sp = (g == nz[b][-1]
                                      and j == c_bg[b][g] - 1)
                                nc.tensor.matmul(pos_[b][:], lhsT=m[:],
                                                 rhs=msg2[:, c, :],
                                                 start=st, stop=sp)
                    for b in blocks:
                        o_sb = work.tile([P, F_out], F32, tag="osb", bufs=3)
                        nc.vector.tensor_tensor(out=o_sb[:], in0=pos_[b][:],
                                                in1=b2_t[:],
                                                op=mybir.AluOpType.add)
                        nc.sync.dma_start(out_d[b * P:(b + 1) * P, :], o_sb[:])
                    si += 1
            psumC.__exit__(None, None, None)

    nc.compile()
    return nc


def _in_maps(cfg, cores, shared):
    return [{**shared, **c} for c in cores]


def _assemble(cfg, outs):
    N, F_out, npc_raw = cfg["N"], cfg["F_out"], cfg["npc_raw"]
    pos = cfg["pos"]
    full = np.empty((N, F_out), np.float32)
    for c in range(NCORES):
        nodes = np.arange(c * npc_raw, (c + 1) * npc_raw)
        full[nodes] = outs[c][pos[nodes]]
    return full


# ---------------------------------------------------------------------------
# entry points
# ---------------------------------------------------------------------------

def kernel(x, edge_index, W1, b1, W2, b2):
    cfg, cores, shared = _prep(x, edge_index, W1, b1, W2, b2)
    nc = _build_nc(cfg)
    from concourse.bass_utils import run_bass_kernel_spmd
    res = run_bass_kernel_spmd(nc, _in_maps(cfg, cores, shared),
                               list(range(NCORES)))
    return _assemble(cfg, [r["out"] for r in res.results])


def run_profiled(x, edge_index, W1, b1, W2, b2, tmpdir=None):
    """Like kernel(), but traces on HW; returns (out, exec_time_ns, tmpdir)."""
    import time

    t0 = time.time()
    cfg, cores, shared = _prep(x, edge_index, W1, b1, W2, b2)
    print(f"prep {time.time() - t0:.1f}s; chunks/phase={int(cfg['c_bg'].sum())} "
          f"nb={cfg['nb']}")
    t0 = time.time()
    nc = _build_nc(cfg)
    print(f"build {time.time() - t0:.1f}s; {len(nc.inst_map)} instructions")
    from concourse.bass_utils import run_bass_kernel_spmd
    in_maps = _in_maps(cfg, cores, shared)
    t0 = time.time()
    res = run_bass_kernel_spmd(nc, in_maps, list(range(NCORES)))
    print(f"run {time.time() - t0:.1f}s")
    out = _assemble(cfg, [r["out"] for r in res.results])
    exec_ns = None
    try:
        t0 = time.time()
        res2 = run_bass_kernel_spmd(nc, in_maps, list(range(NCORES)),
                                    trace=True, tmpdir=tmpdir)
        print(f"traced run {time.time() - t0:.1f}s")
        exec_ns = res2.exec_time_ns
    except Exception as e:
        print(f"trace run failed: {type(e).__name__}: {str(e)[:200]}")
    return out, exec_ns, tmpdir


def _numpy_ref(x, edge_index, W1, b1, W2, b2):
    N = x.shape[0]
    src = np.concatenate([edge_index[0], np.arange(N)])
    dst = np.concatenate([edge_index[1], np.arange(N)])
    deg = np.bincount(dst, minlength=N).astype(np.float64)
    dinv = np.where(deg > 0, 1 / np.sqrt(deg), 0)
    nrm = (dinv[src] * dinv[dst]).astype(np.float32)

    def layer(h, W, b):
        hw = h @ W
        out = np.zeros((N, W.shape[1]), np.float32)
        np.add.at(out, dst, hw[src] * nrm[:, None])
        return out + b

    h = np.maximum(layer(x, W1, b1), 0)
    return layer(h, W2, b2)


def _selftest_sim():
    from concourse import bass_interp
    rng = np.random.default_rng(1)
    N, E, F_in = 2048, 8192, 512
    x = rng.standard_normal((N, F_in), dtype=np.float32)
    ei = rng.integers(0, N, (2, E)).astype(np.int64)
    W1 = (rng.standard_normal((F_in, 256), dtype=np.float32) * F_in ** -0.5)
    W2 = (rng.standard_normal((256, 128), dtype=np.float32) * 256 ** -0.5)
    b1 = rng.standard_normal(256).astype(np.float32) * 0.1
    b2 = rng.standard_normal(128).astype(np.float32) * 0.1

    cfg, cores, shared = _prep(x, ei, W1, b1, W2, b2)
    print("cfg:", {k: (v if k != "c_bg" else v.tolist())
                   for k, v in cfg.items() if k != "pos"})
    nc = _build_nc(cfg)
    print("built; instructions:", len(nc.inst_map))

    sim = bass_interp.MultiCoreSim(nc, NCORES)
    for i, m in enumerate(_in_maps(cfg, cores, shared)):
        for k, v in m.items():
            sim.cores[i].tensor(k)[:] = v
    sim.simulate()
    outs = [np.array(sim.cores[i].mem_tensor("out")) for i in range(NCORES)]
    got = _assemble(cfg, outs)
    want = _numpy_ref(x, ei, W1, b1, W2, b2)
    err = np.abs(got - want).max() / (np.abs(want).max() + 1e-9)
    print("selftest rel err:", err)
    assert err < 1e-2, "selftest FAILED"
    print("SELFTEST PASSED")


if __name__ == "__main__":
    _selftest_sim()
